# revision 8
# baseline (speedup 1.0000x reference)
import numpy as np
import ml_dtypes

bf16 = ml_dtypes.bfloat16

H = 12
HS = 64
ALL = H * HS          # 768
P = 128
B = 2
S = 1024
C = 64                # output channels (W_out cols)
SCALING = HS ** 0.25  # 2.8284...
S_CORE = 256          # s-rows per core
NSLAB = S_CORE // 8   # 32 slabs of 8 s-rows
NCORES = 8

# D (gather data) layout: [pad0 x3 | qeb[0..254] | pad254 x3] = 261, padded to 264
D_W = 264

_COMPILED = None


def _build_nc():
    import concourse.bacc as bacc
    import concourse.mybir as mybir
    from concourse.tile import TileContext

    dt = mybir.dt
    AF = mybir.ActivationFunctionType
    ALU = mybir.AluOpType

    nc = bacc.Bacc()

    p1T = nc.dram_tensor("p1T", [ALL, S], dt.bfloat16, kind="ExternalInput")
    p1Tq = nc.dram_tensor("p1Tq", [ALL, S_CORE], dt.bfloat16, kind="ExternalInput")
    wqk = nc.dram_tensor("wqk", [ALL, 2 * ALL], dt.bfloat16, kind="ExternalInput")
    b2d = nc.dram_tensor("b2d", [128, 12], dt.float32, kind="ExternalInput")
    relT = nc.dram_tensor("relT", [128, 256], dt.bfloat16, kind="ExternalInput")
    wbd = nc.dram_tensor("wbd", [128, 4, 128], dt.bfloat16, kind="ExternalInput")
    bout2 = nc.dram_tensor("bout2", [128, 1], dt.float32, kind="ExternalInput")
    ident = nc.dram_tensor("ident", [128, 128], dt.bfloat16, kind="ExternalInput")
    idxs = nc.dram_tensor("idxs", [128, 16 * NSLAB], dt.uint16, kind="ExternalInput")

    out = nc.dram_tensor("out", [S_CORE, S, C], dt.bfloat16, kind="ExternalOutput")

    inv_s = float(1.0 / SCALING)

    with TileContext(nc) as tc:
        with (
            tc.tile_pool(name="const", bufs=1) as cpool,
            tc.tile_pool(name="persist", bufs=1) as ppool,
            tc.tile_pool(name="slab", bufs=2) as spool,
            tc.tile_pool(name="outp", bufs=3) as opool,
        ):
            # ---- constant loads ----
            wqk_t = cpool.tile([128, 6, 2 * ALL], dt.bfloat16)
            nc.sync.dma_start(out=wqk_t[:], in_=wqk.rearrange("(a p) f -> p a f", p=128))
            p1T_t = cpool.tile([128, 6, S], dt.bfloat16)
            nc.sync.dma_start(out=p1T_t[:], in_=p1T.rearrange("(a p) s -> p a s", p=128))
            p1Tq_t = cpool.tile([128, 6, S_CORE], dt.bfloat16)
            nc.sync.dma_start(out=p1Tq_t[:], in_=p1Tq.rearrange("(a p) s -> p a s", p=128))
            b2d_t = cpool.tile([128, 12], dt.float32)
            nc.sync.dma_start(out=b2d_t[:], in_=b2d[:])
            relT_t = cpool.tile([128, 256], dt.bfloat16)
            nc.sync.dma_start(out=relT_t[:], in_=relT[:])
            wbd_t = cpool.tile([128, 4, 128], dt.bfloat16)
            nc.sync.dma_start(out=wbd_t[:], in_=wbd[:])
            bout2_t = cpool.tile([128, 1], dt.float32)
            nc.sync.dma_start(out=bout2_t[:], in_=bout2[:])
            ident_t = cpool.tile([128, 128], dt.bfloat16)
            nc.sync.dma_start(out=ident_t[:], in_=ident[:])
            idxs_t = cpool.tile([128, 16 * NSLAB], dt.uint16)
            nc.sync.dma_start(out=idxs_t[:], in_=idxs[:])

            # ---- projections: uT = W_qk^T-contract(p1T); kT (all S), qT (own S_CORE) ----
            kT_t = ppool.tile([128, 6, S], dt.bfloat16)    # rows f = 768 + cf*128 + p
            qT_t = ppool.tile([128, 6, S_CORE], dt.bfloat16)
            with tc.tile_pool(name="ps_proj", bufs=2, space="PSUM") as pj_pool:
                for cf in range(6):
                    for th in range(2):
                        pj = pj_pool.tile([128, 512], dt.float32)
                        for ca in range(6):
                            nc.tensor.matmul(
                                pj[:],
                                lhsT=wqk_t[:, ca, ALL + cf * 128 : ALL + cf * 128 + 128],
                                rhs=p1T_t[:, ca, th * 512 : th * 512 + 512],
                                start=(ca == 0),
                                stop=(ca == 5),
                            )
                        nc.scalar.activation(
                            kT_t[:, cf, th * 512 : th * 512 + 512],
                            pj[:],
                            AF.Identity,
                            bias=b2d_t[:, 6 + cf : 7 + cf],
                            scale=inv_s,
                        )
                for cf in range(6):
                    pj = pj_pool.tile([128, 512], dt.float32)
                    for ca in range(6):
                        nc.tensor.matmul(
                            pj[:, 0:S_CORE],
                            lhsT=wqk_t[:, ca, cf * 128 : cf * 128 + 128],
                            rhs=p1Tq_t[:, ca, :],
                            start=(ca == 0),
                            stop=(ca == 5),
                        )
                    nc.scalar.activation(
                        qT_t[:, cf, :],
                        pj[:, 0:S_CORE],
                        AF.Identity,
                        bias=b2d_t[:, cf : cf + 1],
                        scale=inv_s,
                    )

            # persistent per-slab weights tiles (double-buffered manually)
            qsl_t = [ppool.tile([128, 128], dt.bfloat16, name=f"qsl{i}", tag=f"qsl{i}") for i in range(2)]
            qbd_t = [ppool.tile([128, 6, 128], dt.bfloat16, name=f"qbd{i}", tag=f"qbd{i}") for i in range(2)]
            for i in range(2):
                nc.vector.memset(qsl_t[i][:], 0.0)
                nc.vector.memset(qbd_t[i][:], 0.0)

            slab_pools = (
                tc.tile_pool(name="ps_qe", bufs=2, space="PSUM"),
                tc.tile_pool(name="ps_a", bufs=2, space="PSUM"),
                tc.tile_pool(name="ps_o", bufs=2, space="PSUM"),
                tc.tile_pool(name="ps_t", bufs=2, space="PSUM"),
            )
            pqe_pool = slab_pools[0].__enter__()
            pa_pool = slab_pools[1].__enter__()
            po_pool = slab_pools[2].__enter__()
            pt_pool = slab_pools[3].__enter__()

            # ---- per-slab pipeline ----
            for g in range(NSLAB):
                s0 = 8 * g
                qsl = qsl_t[g % 2]
                qbd = qbd_t[g % 2]

                # qe lhsT: qsl[d, grp*16+h] = qT[h*64+d, s0+grp]
                qsl_v = qsl.rearrange("p (a b) -> p a b", b=16)
                for h in range(H):
                    r0 = (h % 2) * 64
                    nc.vector.tensor_copy(
                        qsl_v[r0 : r0 + 64, :, h],
                        qT_t[r0 : r0 + 64, h // 2, s0 : s0 + 8],
                    )

                # qe matmul -> [128=(grp,h), 256]
                pqe = pqe_pool.tile([128, 256], dt.float32)
                nc.tensor.matmul(pqe[:], lhsT=qsl[:], rhs=relT_t[:], start=True, stop=True)

                # gather data row: [pad0 x3 | qe[129..255] | qe[0..127] | pad254 x3]
                d_t = spool.tile([128, D_W], dt.bfloat16, tag="dgat")
                nc.scalar.activation(d_t[:, 3:130], pqe[:, 129:256], AF.Copy)
                nc.scalar.activation(d_t[:, 130:258], pqe[:, 0:128], AF.Copy)
                for i in range(3):
                    nc.vector.tensor_copy(d_t[:, i : i + 1], pqe[:, 129:130])
                    nc.vector.tensor_copy(d_t[:, 258 + i : 259 + i], pqe[:, 127:128])

                # bias expansion gather: Bsl[p, t] = D[p, idx(t)+0..3]
                bsl = spool.tile([128, S], dt.bfloat16, tag="bsl")
                nc.gpsimd.indirect_copy(
                    bsl.rearrange("p (n i) -> p n i", i=4),
                    d_t.rearrange("p (n i) -> p n i", i=4),
                    idxs_t[:, g * 16 : g * 16 + 16],
                    True,
                )

                # block-sparse q weights: qbd[c][(h',d) rows, grp*16+h] = qT
                qbd_v = qbd.rearrange("p k (a b) -> p k a b", b=16)
                for c6 in range(6):
                    nc.vector.tensor_copy(
                        qbd_v[0:64, c6, :, 2 * c6],
                        qT_t[0:64, c6, s0 : s0 + 8],
                    )
                    nc.vector.tensor_copy(
                        qbd_v[64:128, c6, :, 2 * c6 + 1],
                        qT_t[64:128, c6, s0 : s0 + 8],
                    )

                # scores + bias add -> A [128=(grp,h), 1024] bf16
                a_t = spool.tile([128, S], dt.bfloat16, tag="a")
                for th in range(2):
                    pa = pa_pool.tile([128, 512], dt.float32)
                    for c6 in range(6):
                        nc.tensor.matmul(
                            pa[:],
                            lhsT=qbd[:, c6, :],
                            rhs=kT_t[:, c6, th * 512 : th * 512 + 512],
                            start=(c6 == 0),
                            stop=(c6 == 5),
                        )
                    nc.vector.tensor_tensor(
                        a_t[:, th * 512 : th * 512 + 512],
                        pa[:],
                        bsl[:, th * 512 : th * 512 + 512],
                        op=ALU.add,
                    )

                # final W_out contraction: out rows (j,c) for s-pair j2
                o1 = spool.tile([128, 4, S], dt.bfloat16, tag="o1")
                for j2 in range(4):
                    for th in range(2):
                        po = po_pool.tile([128, 512], dt.float32)
                        nc.tensor.matmul(
                            po[:],
                            lhsT=wbd_t[:, j2, :],
                            rhs=a_t[:, th * 512 : th * 512 + 512],
                            start=True,
                            stop=True,
                        )
                        nc.scalar.activation(
                            o1[:, j2, th * 512 : th * 512 + 512],
                            po[:],
                            AF.Identity,
                            bias=bout2_t[:],
                        )

                # transpose to [t, (s,c)] and store
                for tc_i in range(8):
                    so = opool.tile([128, 8, 64], dt.bfloat16, tag="so")
                    for j2 in range(4):
                        pt = pt_pool.tile([128, 128], dt.bfloat16)
                        nc.tensor.transpose(
                            pt[:], o1[:, j2, tc_i * 128 : tc_i * 128 + 128], ident_t[:]
                        )
                        nc.vector.tensor_copy(
                            so[:, 2 * j2 : 2 * j2 + 2, :].rearrange("p a b -> p (a b)"),
                            pt[:],
                        )
                    nc.sync.dma_start(
                        out=out[s0 : s0 + 8, tc_i * 128 : tc_i * 128 + 128, :].rearrange(
                            "s t c -> t s c"
                        ),
                        in_=so[:],
                    )

            for cm in reversed(slab_pools):
                cm.__exit__(None, None, None)

    nc.finalize()
    return nc


def _host_prep(p1, W_qk, b_qk, rel_emb, W_out, b_out):
    wqk_bf = W_qk.astype(bf16)
    b2d = np.ascontiguousarray((b_qk / SCALING).reshape(12, 128).T.astype(np.float32))
    relT1 = np.ascontiguousarray((rel_emb[:256] / SCALING).T).astype(bf16)
    relT = np.concatenate([relT1, relT1], axis=0)
    wbd = np.zeros((128, 4, 128), np.float32)
    for j2 in range(4):
        for j in range(2):
            grp = 2 * j2 + j
            wbd[grp * 16 : grp * 16 + H, j2, j * 64 : j * 64 + 64] = W_out
    wbd = wbd.astype(bf16)
    bout2 = np.tile(np.asarray(b_out, np.float32), 2)[:, None]
    bout2 = np.ascontiguousarray(bout2)
    ident = np.eye(128, dtype=np.float32).astype(bf16)

    p1T = [np.ascontiguousarray(p1[b].T).astype(bf16) for b in range(B)]

    in_maps = []
    for core in range(NCORES):
        b = core // 4
        s_off = (core % 4) * S_CORE
        p1Tq = np.ascontiguousarray(p1T[b][:, s_off : s_off + S_CORE])

        # gather index table: [128, 16*NSLAB] uint16
        idxs = np.zeros((128, 16 * NSLAB), np.uint16)
        i_arr = np.arange(256)
        for g in range(NSLAB):
            for grp in range(8):
                s_glob = s_off + 8 * g + grp
                j = 4 * i_arr - s_glob + 127
                idx = np.clip(j, -3, 254) + 3
                idxs[16 * grp + (i_arr % 16), g * 16 + i_arr // 16] = idx.astype(
                    np.uint16
                )

        in_maps.append(
            {
                "p1T": p1T[b],
                "p1Tq": p1Tq,
                "wqk": wqk_bf,
                "b2d": b2d,
                "relT": relT,
                "wbd": wbd,
                "bout2": bout2,
                "ident": ident,
                "idxs": idxs,
            }
        )
    return in_maps


def kernel(p0, p1, p2, W_qk, b_qk, rel_emb, W_out, b_out):
    global _COMPILED
    from concourse.bass_utils import run_bass_kernel_spmd

    p1 = np.asarray(p1, np.float32)
    W_qk = np.asarray(W_qk, np.float32)
    b_qk = np.asarray(b_qk, np.float32)
    rel_emb = np.asarray(rel_emb, np.float32)
    W_out = np.asarray(W_out, np.float32)
    b_out = np.asarray(b_out, np.float32)

    if _COMPILED is None:
        _COMPILED = _build_nc()
    nc = _COMPILED

    in_maps = _host_prep(p1, W_qk, b_qk, rel_emb, W_out, b_out)
    res = run_bass_kernel_spmd(nc, in_maps, list(range(NCORES)))

    full = np.empty((B, S, S, C), np.float32)
    for core in range(NCORES):
        b = core // 4
        s_off = (core % 4) * S_CORE
        full[b, s_off : s_off + S_CORE] = res.results[core]["out"].astype(np.float32)
    return full


# revision 9
# speedup vs baseline: 1.7573x; 1.7573x over previous
import numpy as np
import ml_dtypes

bf16 = ml_dtypes.bfloat16

H = 12
HS = 64
ALL = H * HS          # 768
P = 128
B = 2
S = 1024
C = 64                # output channels (W_out cols)
SCALING = HS ** 0.25  # 2.8284...
S_CORE = 256          # s-rows per core
NSLAB = S_CORE // 8   # 32 slabs of 8 s-rows
NCORES = 8

# D (gather data) layout: [pad0 x3 | qeb[0..254] | pad254 x3] = 261, padded to 264
D_W = 264

_COMPILED = None


def _build_nc():
    import concourse.bacc as bacc
    import concourse.mybir as mybir
    from concourse.tile import TileContext

    dt = mybir.dt
    AF = mybir.ActivationFunctionType
    ALU = mybir.AluOpType

    nc = bacc.Bacc()

    p1T = nc.dram_tensor("p1T", [ALL, S], dt.bfloat16, kind="ExternalInput")
    p1Tq = nc.dram_tensor("p1Tq", [ALL, S_CORE], dt.bfloat16, kind="ExternalInput")
    wqk = nc.dram_tensor("wqk", [ALL, 2 * ALL], dt.bfloat16, kind="ExternalInput")
    b2d = nc.dram_tensor("b2d", [128, 12], dt.float32, kind="ExternalInput")
    relT = nc.dram_tensor("relT", [128, 256], dt.bfloat16, kind="ExternalInput")
    wbd = nc.dram_tensor("wbd", [128, 4, 128], dt.bfloat16, kind="ExternalInput")
    bout2 = nc.dram_tensor("bout2", [128, 1], dt.float32, kind="ExternalInput")
    ident = nc.dram_tensor("ident", [128, 128], dt.bfloat16, kind="ExternalInput")
    idxs = nc.dram_tensor("idxs", [128, 16 * NSLAB], dt.uint16, kind="ExternalInput")

    out = nc.dram_tensor("out", [S_CORE, S, C], dt.bfloat16, kind="ExternalOutput")

    inv_s = float(1.0 / SCALING)

    with TileContext(nc) as tc:
        with (
            tc.tile_pool(name="const", bufs=1) as cpool,
            tc.tile_pool(name="persist", bufs=1) as ppool,
            tc.tile_pool(name="slab", bufs=2) as spool,
            tc.tile_pool(name="outp", bufs=3) as opool,
        ):
            # ---- constant loads ----
            wqk_t = cpool.tile([128, 6, 2 * ALL], dt.bfloat16)
            nc.sync.dma_start(out=wqk_t[:], in_=wqk.rearrange("(a p) f -> p a f", p=128))
            p1T_t = cpool.tile([128, 6, S], dt.bfloat16)
            nc.sync.dma_start(out=p1T_t[:], in_=p1T.rearrange("(a p) s -> p a s", p=128))
            p1Tq_t = cpool.tile([128, 6, S_CORE], dt.bfloat16)
            nc.sync.dma_start(out=p1Tq_t[:], in_=p1Tq.rearrange("(a p) s -> p a s", p=128))
            b2d_t = cpool.tile([128, 12], dt.float32)
            nc.sync.dma_start(out=b2d_t[:], in_=b2d[:])
            relT_t = cpool.tile([128, 256], dt.bfloat16)
            nc.sync.dma_start(out=relT_t[:], in_=relT[:])
            wbd_t = cpool.tile([128, 4, 128], dt.bfloat16)
            nc.sync.dma_start(out=wbd_t[:], in_=wbd[:])
            bout2_t = cpool.tile([128, 1], dt.float32)
            nc.sync.dma_start(out=bout2_t[:], in_=bout2[:])
            ident_t = cpool.tile([128, 128], dt.bfloat16)
            nc.sync.dma_start(out=ident_t[:], in_=ident[:])
            idxs_t = cpool.tile([128, 16 * NSLAB], dt.uint16)
            nc.sync.dma_start(out=idxs_t[:], in_=idxs[:])

            # ---- projections: uT = W_qk^T-contract(p1T); kT (all S), qT (own S_CORE) ----
            kT_t = ppool.tile([128, 6, S], dt.bfloat16)    # rows f = 768 + cf*128 + p
            qT_t = ppool.tile([128, 6, S_CORE], dt.bfloat16)
            with tc.tile_pool(name="ps_proj", bufs=2, space="PSUM") as pj_pool:
                for cf in range(6):
                    for th in range(2):
                        pj = pj_pool.tile([128, 512], dt.float32)
                        for ca in range(6):
                            nc.tensor.matmul(
                                pj[:],
                                lhsT=wqk_t[:, ca, ALL + cf * 128 : ALL + cf * 128 + 128],
                                rhs=p1T_t[:, ca, th * 512 : th * 512 + 512],
                                start=(ca == 0),
                                stop=(ca == 5),
                            )
                        nc.scalar.activation(
                            kT_t[:, cf, th * 512 : th * 512 + 512],
                            pj[:],
                            AF.Identity,
                            bias=b2d_t[:, 6 + cf : 7 + cf],
                            scale=inv_s,
                        )
                for cf in range(6):
                    pj = pj_pool.tile([128, 512], dt.float32)
                    for ca in range(6):
                        nc.tensor.matmul(
                            pj[:, 0:S_CORE],
                            lhsT=wqk_t[:, ca, cf * 128 : cf * 128 + 128],
                            rhs=p1Tq_t[:, ca, :],
                            start=(ca == 0),
                            stop=(ca == 5),
                        )
                    nc.scalar.activation(
                        qT_t[:, cf, :],
                        pj[:, 0:S_CORE],
                        AF.Identity,
                        bias=b2d_t[:, cf : cf + 1],
                        scale=inv_s,
                    )

            # persistent per-slab weights tiles (double-buffered manually)
            qsl_t = [ppool.tile([128, 128], dt.bfloat16, name=f"qsl{i}", tag=f"qsl{i}") for i in range(2)]
            qbd_t = [ppool.tile([128, 6, 128], dt.bfloat16, name=f"qbd{i}", tag=f"qbd{i}") for i in range(2)]
            for i in range(2):
                nc.vector.memset(qsl_t[i][:], 0.0)
                nc.vector.memset(qbd_t[i][:], 0.0)

            slab_pools = (
                tc.tile_pool(name="ps_qe", bufs=2, space="PSUM"),
                tc.tile_pool(name="ps_a", bufs=2, space="PSUM"),
                tc.tile_pool(name="ps_o", bufs=2, space="PSUM"),
                tc.tile_pool(name="ps_t", bufs=2, space="PSUM"),
            )
            pqe_pool = slab_pools[0].__enter__()
            pa_pool = slab_pools[1].__enter__()
            po_pool = slab_pools[2].__enter__()
            pt_pool = slab_pools[3].__enter__()

            # ---- per-slab pipeline ----
            for g in range(NSLAB):
                s0 = 8 * g
                qsl = qsl_t[g % 2]
                qbd = qbd_t[g % 2]

                # qe lhsT: qsl[d, grp*16+h] = qT[h*64+d, s0+grp]
                qsl_v = qsl.rearrange("p (a b) -> p a b", b=16)
                for h in range(H):
                    r0 = (h % 2) * 64
                    nc.vector.tensor_copy(
                        qsl_v[r0 : r0 + 64, :, h],
                        qT_t[r0 : r0 + 64, h // 2, s0 : s0 + 8],
                    )

                # qe matmul -> [128=(grp,h), 256]
                pqe = pqe_pool.tile([128, 256], dt.float32)
                nc.tensor.matmul(pqe[:], lhsT=qsl[:], rhs=relT_t[:], start=True, stop=True)

                # gather data row: [pad0 x3 | qe[129..255] | qe[0..127] | pad254 x3]
                d_t = spool.tile([128, D_W], dt.bfloat16, tag="dgat")
                nc.scalar.activation(d_t[:, 3:130], pqe[:, 129:256], AF.Copy)
                nc.scalar.activation(d_t[:, 130:258], pqe[:, 0:128], AF.Copy)
                for i in range(3):
                    nc.vector.tensor_copy(d_t[:, i : i + 1], pqe[:, 129:130])
                    nc.vector.tensor_copy(d_t[:, 258 + i : 259 + i], pqe[:, 127:128])

                # bias expansion gather: Bsl[p, t] = D[p, idx(t)+0..3]
                bsl = spool.tile([128, S], dt.bfloat16, tag="bsl")
                nc.gpsimd.indirect_copy(
                    bsl.rearrange("p (n i) -> p n i", i=4),
                    d_t.rearrange("p (n i) -> p n i", i=4),
                    idxs_t[:, g * 16 : g * 16 + 16],
                    True,
                )

                # block-sparse q weights: qbd[c][(h',d) rows, grp*16+h] = qT
                qbd_v = qbd.rearrange("p k (a b) -> p k a b", b=16)
                for c6 in range(6):
                    nc.vector.tensor_copy(
                        qbd_v[0:64, c6, :, 2 * c6],
                        qT_t[0:64, c6, s0 : s0 + 8],
                    )
                    nc.vector.tensor_copy(
                        qbd_v[64:128, c6, :, 2 * c6 + 1],
                        qT_t[64:128, c6, s0 : s0 + 8],
                    )

                # scores + bias add -> A [128=(grp,h), 1024] bf16
                a_t = spool.tile([128, S], dt.bfloat16, tag="a")
                for th in range(2):
                    pa = pa_pool.tile([128, 512], dt.float32)
                    for c6 in range(6):
                        nc.tensor.matmul(
                            pa[:],
                            lhsT=qbd[:, c6, :],
                            rhs=kT_t[:, c6, th * 512 : th * 512 + 512],
                            start=(c6 == 0),
                            stop=(c6 == 5),
                        )
                    nc.vector.tensor_tensor(
                        a_t[:, th * 512 : th * 512 + 512],
                        pa[:],
                        bsl[:, th * 512 : th * 512 + 512],
                        op=ALU.add,
                    )

                # final W_out contraction: out rows (j,c) for s-pair j2
                o1 = spool.tile([128, 4, S], dt.bfloat16, tag="o1")
                for j2 in range(4):
                    for th in range(2):
                        po = po_pool.tile([128, 512], dt.float32)
                        nc.tensor.matmul(
                            po[:],
                            lhsT=wbd_t[:, j2, :],
                            rhs=a_t[:, th * 512 : th * 512 + 512],
                            start=True,
                            stop=True,
                        )
                        nc.scalar.activation(
                            o1[:, j2, th * 512 : th * 512 + 512],
                            po[:],
                            AF.Identity,
                            bias=bout2_t[:],
                        )

                # transpose to [t, (s,c)] and store
                for tc_i in range(8):
                    so = opool.tile([128, 8, 64], dt.bfloat16, tag="so")
                    for j2 in range(4):
                        pt = pt_pool.tile([128, 128], dt.bfloat16)
                        nc.tensor.transpose(
                            pt[:], o1[:, j2, tc_i * 128 : tc_i * 128 + 128], ident_t[:]
                        )
                        nc.vector.tensor_copy(
                            so[:, 2 * j2 : 2 * j2 + 2, :].rearrange("p a b -> p (a b)"),
                            pt[:],
                        )
                    nc.sync.dma_start(
                        out=out[s0 : s0 + 8, tc_i * 128 : tc_i * 128 + 128, :].rearrange(
                            "s t c -> t s c"
                        ),
                        in_=so[:],
                    )

            for cm in reversed(slab_pools):
                cm.__exit__(None, None, None)

    nc.finalize()
    return nc


def _host_prep(p1, W_qk, b_qk, rel_emb, W_out, b_out):
    wqk_bf = W_qk.astype(bf16)
    b2d = np.ascontiguousarray((b_qk / SCALING).reshape(12, 128).T.astype(np.float32))
    relT1 = np.ascontiguousarray((rel_emb[:256] / SCALING).T).astype(bf16)
    relT = np.concatenate([relT1, relT1], axis=0)
    wbd = np.zeros((128, 4, 128), np.float32)
    for j2 in range(4):
        for j in range(2):
            grp = 2 * j2 + j
            wbd[grp * 16 : grp * 16 + H, j2, j * 64 : j * 64 + 64] = W_out
    wbd = wbd.astype(bf16)
    bout2 = np.tile(np.asarray(b_out, np.float32), 2)[:, None]
    bout2 = np.ascontiguousarray(bout2)
    ident = np.eye(128, dtype=np.float32).astype(bf16)

    p1T = [np.ascontiguousarray(p1[b].T).astype(bf16) for b in range(B)]

    in_maps = []
    for core in range(NCORES):
        b = core // 4
        s_off = (core % 4) * S_CORE
        p1Tq = np.ascontiguousarray(p1T[b][:, s_off : s_off + S_CORE])

        # gather index table: [128, 16*NSLAB] uint16
        idxs = np.zeros((128, 16 * NSLAB), np.uint16)
        i_arr = np.arange(256)
        for g in range(NSLAB):
            for grp in range(8):
                s_glob = s_off + 8 * g + grp
                j = 4 * i_arr - s_glob + 127
                idx = np.clip(j, -3, 254) + 3
                idxs[16 * grp + (i_arr % 16), g * 16 + i_arr // 16] = idx.astype(
                    np.uint16
                )

        in_maps.append(
            {
                "p1T": p1T[b],
                "p1Tq": p1Tq,
                "wqk": wqk_bf,
                "b2d": b2d,
                "relT": relT,
                "wbd": wbd,
                "bout2": bout2,
                "ident": ident,
                "idxs": idxs,
            }
        )
    return in_maps


def _make_runner():
    """Build the bass module once and return a callable(in_maps) -> list of
    per-core output arrays, with a persistently cached jitted executable."""
    import jax
    import jax.numpy as jnp
    from jax.experimental.shard_map import shard_map
    from jax.sharding import Mesh, NamedSharding, PartitionSpec

    from concourse import bass2jax
    from concourse.bass2jax import _bass_exec_p, install_neuronx_cc_hook

    install_neuronx_cc_hook()
    nc = _build_nc()

    in_names = ["p1T", "p1Tq", "wqk", "b2d", "relT", "wbd", "bout2", "ident", "idxs"]
    out_name = "out"
    out_shape = (S_CORE, S, C)
    partition_name = nc.partition_id_tensor.name if nc.partition_id_tensor else None
    out_aval = jax.core.ShapedArray(out_shape, np.dtype(bf16))

    all_in_names = list(in_names) + [out_name]
    if partition_name is not None:
        all_in_names.append(partition_name)

    def _body(*args):
        operands = list(args)
        if partition_name is not None:
            operands.append(bass2jax.partition_id_tensor())
        outs = _bass_exec_p.bind(
            *operands,
            out_avals=(out_aval,),
            in_names=tuple(all_in_names),
            out_names=(out_name,),
            lowering_input_output_aliases=(),
            sim_require_finite=True,
            sim_require_nnan=True,
            nc=nc,
        )
        return tuple(outs)

    devices = jax.devices()[:NCORES]
    mesh = Mesh(np.asarray(devices), ("core",))
    n_params = len(in_names)
    in_specs = (PartitionSpec("core"),) * (n_params + 1)
    out_specs = (PartitionSpec("core"),)
    sharded = jax.jit(
        shard_map(_body, mesh=mesh, in_specs=in_specs, out_specs=out_specs,
                  check_rep=False),
        donate_argnums=(n_params,),
        keep_unused=True,
    )
    sharding = NamedSharding(mesh, PartitionSpec("core"))
    zeros_fn = jax.jit(
        lambda: jnp.zeros((NCORES * S_CORE, S, C), np.dtype(bf16)),
        out_shardings=sharding,
    )

    def run(in_maps):
        concat_in = [
            np.concatenate([np.asarray(m[name]) for m in in_maps], axis=0)
            for name in in_names
        ]
        zero_out = zeros_fn()
        out_arr = sharded(*concat_in, zero_out)[0]
        out_np = np.asarray(out_arr)
        return out_np.reshape(NCORES, S_CORE, S, C)

    return run


def kernel(p0, p1, p2, W_qk, b_qk, rel_emb, W_out, b_out):
    global _COMPILED

    p1 = np.asarray(p1, np.float32)
    W_qk = np.asarray(W_qk, np.float32)
    b_qk = np.asarray(b_qk, np.float32)
    rel_emb = np.asarray(rel_emb, np.float32)
    W_out = np.asarray(W_out, np.float32)
    b_out = np.asarray(b_out, np.float32)

    if _COMPILED is None:
        _COMPILED = _make_runner()
    run = _COMPILED

    in_maps = _host_prep(p1, W_qk, b_qk, rel_emb, W_out, b_out)
    shards = run(in_maps)

    full = np.empty((B, S, S, C), np.float32)
    for core in range(NCORES):
        b = core // 4
        s_off = (core % 4) * S_CORE
        full[b, s_off : s_off + S_CORE] = shards[core].astype(np.float32)
    return full


# revision 13
# speedup vs baseline: 1.8196x; 1.0354x over previous
import numpy as np
import ml_dtypes

bf16 = ml_dtypes.bfloat16

H = 12
HS = 64
ALL = H * HS          # 768
P = 128
B = 2
S = 1024
C = 64                # output channels (W_out cols)
SCALING = HS ** 0.25  # 2.8284...
S_CORE = 256          # s-rows per core
NSLAB = S_CORE // 8   # 32 slabs of 8 s-rows
NCORES = 8

# D (gather data) layout: [pad0 x3 | qeb[0..254] | pad254 x3] = 261, padded to 264
D_W = 264

_COMPILED = None


def _build_nc(nslab=NSLAB):
    import concourse.bacc as bacc
    import concourse.mybir as mybir
    from concourse.tile import TileContext

    dt = mybir.dt
    AF = mybir.ActivationFunctionType
    ALU = mybir.AluOpType

    nc = bacc.Bacc()

    p1T = nc.dram_tensor("p1T", [ALL, S], dt.bfloat16, kind="ExternalInput")
    p1Tq = nc.dram_tensor("p1Tq", [ALL, S_CORE], dt.bfloat16, kind="ExternalInput")
    wqk = nc.dram_tensor("wqk", [ALL, 2 * ALL], dt.bfloat16, kind="ExternalInput")
    b2d = nc.dram_tensor("b2d", [128, 12], dt.float32, kind="ExternalInput")
    relT = nc.dram_tensor("relT", [128, 256], dt.bfloat16, kind="ExternalInput")
    wbd = nc.dram_tensor("wbd", [128, 4, 128], dt.bfloat16, kind="ExternalInput")
    bout2 = nc.dram_tensor("bout2", [128, 1], dt.float32, kind="ExternalInput")
    ident = nc.dram_tensor("ident", [128, 128], dt.bfloat16, kind="ExternalInput")
    idxs = nc.dram_tensor("idxs", [128, 16 * NSLAB], dt.uint16, kind="ExternalInput")

    out = nc.dram_tensor("out", [S_CORE, S, C], dt.bfloat16, kind="ExternalOutput")

    inv_s = float(1.0 / SCALING)

    with TileContext(nc) as tc:
        with (
            tc.tile_pool(name="const", bufs=1) as cpool,
            tc.tile_pool(name="persist", bufs=1) as ppool,
            tc.tile_pool(name="slab", bufs=2) as spool,
            tc.tile_pool(name="outp", bufs=3) as opool,
        ):
            # ---- constant loads ----
            wqk_t = cpool.tile([128, 6, 2 * ALL], dt.bfloat16)
            nc.sync.dma_start(out=wqk_t[:], in_=wqk.rearrange("(a p) f -> p a f", p=128))
            p1T_t = cpool.tile([128, 6, S], dt.bfloat16)
            nc.sync.dma_start(out=p1T_t[:], in_=p1T.rearrange("(a p) s -> p a s", p=128))
            p1Tq_t = cpool.tile([128, 6, S_CORE], dt.bfloat16)
            nc.sync.dma_start(out=p1Tq_t[:], in_=p1Tq.rearrange("(a p) s -> p a s", p=128))
            b2d_t = cpool.tile([128, 12], dt.float32)
            nc.sync.dma_start(out=b2d_t[:], in_=b2d[:])
            relT_t = cpool.tile([128, 256], dt.bfloat16)
            nc.sync.dma_start(out=relT_t[:], in_=relT[:])
            wbd_t = cpool.tile([128, 4, 128], dt.bfloat16)
            nc.sync.dma_start(out=wbd_t[:], in_=wbd[:])
            bout2_t = cpool.tile([128, 1], dt.float32)
            nc.sync.dma_start(out=bout2_t[:], in_=bout2[:])
            ident_t = cpool.tile([128, 128], dt.bfloat16)
            nc.sync.dma_start(out=ident_t[:], in_=ident[:])
            idxs_t = cpool.tile([128, 16 * NSLAB], dt.uint16)
            nc.sync.dma_start(out=idxs_t[:], in_=idxs[:])

            # ---- projections: uT = W_qk^T-contract(p1T); kT (all S), qT (own S_CORE) ----
            kT_t = ppool.tile([128, 6, S], dt.bfloat16)    # rows f = 768 + cf*128 + p
            qT_t = ppool.tile([128, 6, S_CORE], dt.bfloat16)
            with tc.tile_pool(name="ps_proj", bufs=2, space="PSUM") as pj_pool:
                for cf in range(6):
                    for th in range(2):
                        pj = pj_pool.tile([128, 512], dt.float32)
                        for ca in range(6):
                            nc.tensor.matmul(
                                pj[:],
                                lhsT=wqk_t[:, ca, ALL + cf * 128 : ALL + cf * 128 + 128],
                                rhs=p1T_t[:, ca, th * 512 : th * 512 + 512],
                                start=(ca == 0),
                                stop=(ca == 5),
                            )
                        nc.scalar.activation(
                            kT_t[:, cf, th * 512 : th * 512 + 512],
                            pj[:],
                            AF.Identity,
                            bias=b2d_t[:, 6 + cf : 7 + cf],
                            scale=inv_s,
                        )
                for cf in range(6):
                    pj = pj_pool.tile([128, 512], dt.float32)
                    for ca in range(6):
                        nc.tensor.matmul(
                            pj[:, 0:S_CORE],
                            lhsT=wqk_t[:, ca, cf * 128 : cf * 128 + 128],
                            rhs=p1Tq_t[:, ca, :],
                            start=(ca == 0),
                            stop=(ca == 5),
                        )
                    nc.scalar.activation(
                        qT_t[:, cf, :],
                        pj[:, 0:S_CORE],
                        AF.Identity,
                        bias=b2d_t[:, cf : cf + 1],
                        scale=inv_s,
                    )

            # persistent per-slab weights tiles (double-buffered manually)
            qsl_t = [ppool.tile([128, 128], dt.bfloat16, name=f"qsl{i}", tag=f"qsl{i}") for i in range(2)]
            qbd_t = [ppool.tile([128, 6, 128], dt.bfloat16, name=f"qbd{i}", tag=f"qbd{i}") for i in range(2)]
            for i in range(2):
                nc.vector.memset(qsl_t[i][:], 0.0)
                nc.vector.memset(qbd_t[i][:], 0.0)

            slab_pools = (
                tc.tile_pool(name="ps_qe", bufs=2, space="PSUM"),
                tc.tile_pool(name="ps_a", bufs=2, space="PSUM"),
                tc.tile_pool(name="ps_o", bufs=2, space="PSUM"),
                tc.tile_pool(name="ps_t", bufs=2, space="PSUM"),
            )
            pqe_pool = slab_pools[0].__enter__()
            pa_pool = slab_pools[1].__enter__()
            po_pool = slab_pools[2].__enter__()
            pt_pool = slab_pools[3].__enter__()

            # ---- per-slab pipeline ----
            for g in range(nslab):
                s0 = 8 * g
                qsl = qsl_t[g % 2]
                qbd = qbd_t[g % 2]

                # qe lhsT: qsl[d, grp*16+h] = qT[h*64+d, s0+grp]
                qsl_v = qsl.rearrange("p (a b) -> p a b", b=16)
                for h in range(H):
                    r0 = (h % 2) * 64
                    nc.vector.tensor_copy(
                        qsl_v[r0 : r0 + 64, :, h],
                        qT_t[r0 : r0 + 64, h // 2, s0 : s0 + 8],
                    )

                # qe matmul -> [128=(grp,h), 256]
                pqe = pqe_pool.tile([128, 256], dt.float32)
                nc.tensor.matmul(pqe[:], lhsT=qsl[:], rhs=relT_t[:], start=True, stop=True)

                # gather data row: [pad0 x3 | qe[129..255] | qe[0..127] | pad254 x3]
                d_t = spool.tile([128, D_W], dt.bfloat16, tag="dgat")
                nc.scalar.activation(d_t[:, 3:130], pqe[:, 129:256], AF.Copy)
                nc.scalar.activation(d_t[:, 130:258], pqe[:, 0:128], AF.Copy)
                for i in range(3):
                    nc.vector.tensor_copy(d_t[:, i : i + 1], pqe[:, 129:130])
                    nc.vector.tensor_copy(d_t[:, 258 + i : 259 + i], pqe[:, 127:128])

                # bias expansion gather: Bsl[p, t] = D[p, idx(t)+0..3]
                bsl = spool.tile([128, S], dt.bfloat16, tag="bsl")
                nc.gpsimd.indirect_copy(
                    bsl.rearrange("p (n i) -> p n i", i=4),
                    d_t.rearrange("p (n i) -> p n i", i=4),
                    idxs_t[:, g * 16 : g * 16 + 16],
                    True,
                )

                # block-sparse q weights: qbd[c][(h',d) rows, grp*16+h] = qT
                qbd_v = qbd.rearrange("p k (a b) -> p k a b", b=16)
                for c6 in range(6):
                    nc.vector.tensor_copy(
                        qbd_v[0:64, c6, :, 2 * c6],
                        qT_t[0:64, c6, s0 : s0 + 8],
                    )
                    nc.vector.tensor_copy(
                        qbd_v[64:128, c6, :, 2 * c6 + 1],
                        qT_t[64:128, c6, s0 : s0 + 8],
                    )

                # scores + bias add -> A [128=(grp,h), 1024] bf16
                a_t = spool.tile([128, S], dt.bfloat16, tag="a")
                for th in range(2):
                    pa = pa_pool.tile([128, 512], dt.float32)
                    for c6 in range(6):
                        nc.tensor.matmul(
                            pa[:],
                            lhsT=qbd[:, c6, :],
                            rhs=kT_t[:, c6, th * 512 : th * 512 + 512],
                            start=(c6 == 0),
                            stop=(c6 == 5),
                        )
                    nc.vector.tensor_tensor(
                        a_t[:, th * 512 : th * 512 + 512],
                        pa[:],
                        bsl[:, th * 512 : th * 512 + 512],
                        op=ALU.add,
                    )

                # final W_out contraction: out rows (j,c) for s-pair j2
                o1 = spool.tile([128, 4, S], dt.bfloat16, tag="o1")
                for j2 in range(4):
                    for th in range(2):
                        po = po_pool.tile([128, 512], dt.float32)
                        nc.tensor.matmul(
                            po[:],
                            lhsT=wbd_t[:, j2, :],
                            rhs=a_t[:, th * 512 : th * 512 + 512],
                            start=True,
                            stop=True,
                        )
                        nc.scalar.activation(
                            o1[:, j2, th * 512 : th * 512 + 512],
                            po[:],
                            AF.Identity,
                            bias=bout2_t[:],
                        )

                # transpose to [t, (s,c)] and store
                for tc_i in range(8):
                    so = opool.tile([128, 8, 64], dt.bfloat16, tag="so")
                    for j2 in range(4):
                        pt = pt_pool.tile([128, 128], dt.bfloat16)
                        nc.tensor.transpose(
                            pt[:], o1[:, j2, tc_i * 128 : tc_i * 128 + 128], ident_t[:]
                        )
                        nc.vector.tensor_copy(
                            so[:, 2 * j2 : 2 * j2 + 2, :].rearrange("p a b -> p (a b)"),
                            pt[:],
                        )
                    nc.sync.dma_start(
                        out=out[s0 : s0 + 8, tc_i * 128 : tc_i * 128 + 128, :].rearrange(
                            "s t c -> t s c"
                        ),
                        in_=so[:],
                    )

            for cm in reversed(slab_pools):
                cm.__exit__(None, None, None)

    nc.finalize()
    return nc


def _host_prep(p1, W_qk, b_qk, rel_emb, W_out, b_out):
    wqk_bf = W_qk.astype(bf16)
    b2d = np.ascontiguousarray((b_qk / SCALING).reshape(12, 128).T.astype(np.float32))
    relT1 = np.ascontiguousarray((rel_emb[:256] / SCALING).T).astype(bf16)
    relT = np.concatenate([relT1, relT1], axis=0)
    wbd = np.zeros((128, 4, 128), np.float32)
    for j2 in range(4):
        for j in range(2):
            grp = 2 * j2 + j
            wbd[grp * 16 : grp * 16 + H, j2, j * 64 : j * 64 + 64] = W_out
    wbd = wbd.astype(bf16)
    bout2 = np.tile(np.asarray(b_out, np.float32), 2)[:, None]
    bout2 = np.ascontiguousarray(bout2)
    ident = np.eye(128, dtype=np.float32).astype(bf16)

    p1T = [np.ascontiguousarray(p1[b].T).astype(bf16) for b in range(B)]

    in_maps = []
    for core in range(NCORES):
        b = core // 4
        s_off = (core % 4) * S_CORE
        p1Tq = np.ascontiguousarray(p1T[b][:, s_off : s_off + S_CORE])

        # gather index table: [128, 16*NSLAB] uint16
        idxs = np.zeros((128, 16 * NSLAB), np.uint16)
        i_arr = np.arange(256)
        for g in range(NSLAB):
            for grp in range(8):
                s_glob = s_off + 8 * g + grp
                j = 4 * i_arr - s_glob + 127
                idx = np.clip(j, -3, 254) + 3
                idxs[16 * grp + (i_arr % 16), g * 16 + i_arr // 16] = idx.astype(
                    np.uint16
                )

        in_maps.append(
            {
                "p1T": p1T[b],
                "p1Tq": p1Tq,
                "wqk": wqk_bf,
                "b2d": b2d,
                "relT": relT,
                "wbd": wbd,
                "bout2": bout2,
                "ident": ident,
                "idxs": idxs,
            }
        )
    return in_maps


def _make_runner():
    """Build the bass module once and return a callable(in_maps) -> list of
    per-core output arrays, with a persistently cached jitted executable."""
    import jax
    import jax.numpy as jnp
    from jax.experimental.shard_map import shard_map
    from jax.sharding import Mesh, NamedSharding, PartitionSpec

    from concourse import bass2jax
    from concourse.bass2jax import _bass_exec_p, install_neuronx_cc_hook

    install_neuronx_cc_hook()
    nc = _build_nc()

    in_names = ["p1T", "p1Tq", "wqk", "b2d", "relT", "wbd", "bout2", "ident", "idxs"]
    out_name = "out"
    out_shape = (S_CORE, S, C)
    partition_name = nc.partition_id_tensor.name if nc.partition_id_tensor else None
    out_aval = jax.core.ShapedArray(out_shape, np.dtype(bf16))

    all_in_names = list(in_names) + [out_name]
    if partition_name is not None:
        all_in_names.append(partition_name)

    def _body(*args):
        operands = list(args)
        if partition_name is not None:
            operands.append(bass2jax.partition_id_tensor())
        outs = _bass_exec_p.bind(
            *operands,
            out_avals=(out_aval,),
            in_names=tuple(all_in_names),
            out_names=(out_name,),
            lowering_input_output_aliases=(),
            sim_require_finite=True,
            sim_require_nnan=True,
            nc=nc,
        )
        return tuple(outs)

    devices = jax.devices()[:NCORES]
    mesh = Mesh(np.asarray(devices), ("core",))
    n_params = len(in_names)
    in_specs = (PartitionSpec("core"),) * (n_params + 1)
    out_specs = (PartitionSpec("core"),)
    sharded = jax.jit(
        shard_map(_body, mesh=mesh, in_specs=in_specs, out_specs=out_specs,
                  check_rep=False),
        donate_argnums=(n_params,),
        keep_unused=True,
    )
    sharding = NamedSharding(mesh, PartitionSpec("core"))
    zeros_fn = jax.jit(
        lambda: jnp.zeros((NCORES * S_CORE, S, C), np.dtype(bf16)),
        out_shardings=sharding,
    )

    def run(in_maps, out_f32):
        """Execute and write fp32 results directly into out_f32 [B,S,S,C]."""
        from concurrent.futures import ThreadPoolExecutor

        concat_in = [
            np.concatenate([np.asarray(m[name]) for m in in_maps], axis=0)
            for name in in_names
        ]
        zero_out = zeros_fn()
        out_arr = sharded(*concat_in, zero_out)[0]

        shards = sorted(out_arr.addressable_shards, key=lambda sh: sh.index[0].start)

        def fetch_convert(core_sh):
            core, sh = core_sh
            raw = np.asarray(sh.data)  # [S_CORE, S, C] bf16 (network-bound)
            b = core // 4
            s_off = (core % 4) * S_CORE
            # fast bf16 -> f32 upcast via bit twiddling
            u = raw.view(np.uint16).astype(np.uint32) << 16
            out_f32[b, s_off : s_off + S_CORE] = u.view(np.float32)

        with ThreadPoolExecutor(4) as ex:
            list(ex.map(fetch_convert, enumerate(shards)))

    return run


def kernel(p0, p1, p2, W_qk, b_qk, rel_emb, W_out, b_out):
    global _COMPILED

    p1 = np.asarray(p1, np.float32)
    W_qk = np.asarray(W_qk, np.float32)
    b_qk = np.asarray(b_qk, np.float32)
    rel_emb = np.asarray(rel_emb, np.float32)
    W_out = np.asarray(W_out, np.float32)
    b_out = np.asarray(b_out, np.float32)

    if _COMPILED is None:
        _COMPILED = _make_runner()
    run = _COMPILED

    in_maps = _host_prep(p1, W_qk, b_qk, rel_emb, W_out, b_out)
    full = np.empty((B, S, S, C), np.float32)
    run(in_maps, full)
    return full
